# revision 14
# baseline (speedup 1.0000x reference)
"""MoE layer (SwiGLU experts, top-2 routing) on 8 Trainium2 NeuronCores.

Strategy (expert parallelism, per the sharding hint):
  - The router (a [N,8] matmul + softmax + top-2, ~0.01% of total FLOPs) is
    computed host-side in float64; it determines the token->expert dispatch.
  - Token dispatch/combine (the "all-to-all") is done host-side: each core e
    receives expert e's weights plus the tokens routed to expert e, padded to
    a uniform capacity C (multiple of 128, same on all cores for SPMD).
  - Each core runs the heavy compute: y = (silu(x@wg) * (x@wu)) @ wd scaled
    by the per-token combine weight. All matmul operands are bf16 (1 cyc/row
    on the PE, same rate as fp32r, but half the HBM/SBUF footprint and FWL
    weight loads); PSUM accumulation stays fp32, well within 2e-2 tolerance.
  - Host scatter-adds each expert's output rows back into the final output.
  - Weights are pre-tiled host-side so every DMA reads >=2KB contiguous per
    partition line; wd (8.4 MB in bf16) is loaded once into SBUF and stays
    resident for all of stage 2.

Device kernel structure (per core):
  Tokens are processed in groups of up to 768 (first group smallest so the
  first matmul's DMA window is short). Stage 1 computes
  hT[f, token] = silu(wg.T x) * (wu.T x) for all F=4096 rows of the group,
  accumulating over D=1024 in PSUM (8 matmuls per 128-row f-tile), with the
  gate/up PSUM banks drained by ScalarE (silu) and VectorE (mul, bf16 out)
  into SBUF. Stage 2 contracts hT over F entirely in PSUM (32-matmul
  accumulation per output tile), applies the combine weight, and streams
  results out. The very last PSUM batch is a single c-tile so the final
  drain exposes only ~1us.
"""

import os
import sys

sys.path.insert(0, "/opt/trn_rl_repo")
import numpy as np
import ml_dtypes

BF16 = ml_dtypes.bfloat16

P = 128
D_MODEL = 1024
D_FF = 4096
N_EXPERTS = 8
TOP_K = 2
G_MAX = 768  # token group size: hT for the group stays in SBUF
N_WARMUP = 32  # PE warmup matmuls: ramp the clock + bridge the first DMAs
N_PRE_FT = 4  # f-tiles of next group's wg/wu prefetched during stage 1

LAST_EXEC_NS = None
_programs = {}


def _ensure_axon_hooks():
    """The agent image's antenv lacks axon_hooks; reconstruct it so
    trace=True works (NTFF profiling via libaxon_pjrt ctypes hook)."""
    import types

    try:
        import antenv.axon_hooks  # noqa: F401

        return
    except ImportError:
        pass
    try:
        import antenv

        mod = types.ModuleType("antenv.axon_hooks")
        _hook = [None]
        mod.set_axon_ntff_profile_hook = lambda h: _hook.__setitem__(0, h)
        mod.get_axon_ntff_profile_hook = lambda: _hook[0]
        sys.modules["antenv.axon_hooks"] = mod
        antenv.axon_hooks = mod
        if "/root/.axon_site" not in sys.path:
            sys.path.insert(0, "/root/.axon_site")
        from trn_agent_boot.trn_boot import _ntff_profile_via_ctypes

        mod.set_axon_ntff_profile_hook(
            _ntff_profile_via_ctypes("/opt/axon/libaxon_pjrt.so")
        )
        import concourse.bass_utils as bu

        bu.upload_artifacts = lambda tmpdir: f"local://{tmpdir}"
    except Exception:
        pass


def _group_sizes(C):
    sizes = []
    rem = C
    while rem > 0:
        if rem >= G_MAX + 512 or rem <= G_MAX:
            take = min(G_MAX, rem)
        else:
            take = rem - 512
        sizes.append(take)
        rem -= take
    sizes.sort()
    return sizes


def _build_program(C):
    import concourse.bacc as bacc
    import concourse.mybir as mybir
    from concourse.tile import TileContext

    fp32 = mybir.dt.float32
    bf16 = mybir.dt.bfloat16
    D, F = D_MODEL, D_FF
    DT, FT = D // P, F // P
    GS = C // P  # total 128-token tiles
    silu_fn = mybir.ActivationFunctionType.Silu
    mult_op = mybir.AluOpType.mult

    nc = bacc.Bacc(
        "TRN2", target_bir_lowering=False, debug=False, num_devices=N_EXPERTS
    )
    # host pre-tiled layouts (see kernel()); startup is DMA-completion-
    # latency bound, so gate+up share one tensor (one DMA per f-tile) and the
    # token slab is a single DMA:
    #   xT[p, :]                 = group-major flat token slabs: for each
    #                              group, [dt, c] row-major (one contiguous
    #                              10-12KB run per partition per group)
    #   wgu[ft, p, dt, 0/1, j]   = w_gate/w_up[dt*128+p, ft*128+j]
    #   wdT[p, ft, d]            = w_down[ft*128+p, d]   (SBUF-resident)
    #   scT[p, g]                = combine_weight[g*128+p]
    xT = nc.dram_tensor("xT", [P, DT * C], bf16, kind="ExternalInput")
    wgu_t = nc.dram_tensor("wgu", [FT, P, DT, 2, P], bf16, kind="ExternalInput")
    wd = nc.dram_tensor("wd", [P, FT, D], bf16, kind="ExternalInput")
    sc = nc.dram_tensor("sc", [P, GS], fp32, kind="ExternalInput")
    y = nc.dram_tensor("y", [C, D], fp32, kind="ExternalOutput")

    xT_ap = xT.ap()
    wgu_ap = wgu_t.ap()
    wd_ap = wd.ap()
    sc_ap = sc.ap()
    y_ap = y.ap()

    # groups of <=768, avoiding tails <512; smallest first so the initial
    # DMA window is short
    sizes = _group_sizes(C)
    groups = []
    g0 = 0
    for gc in sizes:
        groups.append((g0, gc))
        g0 += gc
    n_groups = len(groups)

    def batch_plan(gsub, peel_tail):
        # one batch if it fits the 6 stage-2 PSUM banks, else split evenly.
        # peel_tail: split a single final c-tile into its own batch so the
        # terminal PSUM drain is one eviction, not gsub of them.
        if peel_tail and gsub > 1:
            head = batch_plan(gsub - 1, False)
            return head + [[gsub - 1]]
        if gsub <= 6:
            return [list(range(gsub))]
        n_b = (gsub + 5) // 6
        base, extra = divmod(gsub, n_b)
        out, s = [], 0
        for i in range(n_b):
            n = base + (1 if i < extra else 0)
            out.append(list(range(s, s + n)))
            s += n
        return out

    with TileContext(nc) as tc:
        with (
            tc.tile_pool(name="warm", bufs=1) as warm_pool,
            tc.tile_pool(name="xg", bufs=2) as xg_pool,
            tc.tile_pool(name="wgu", bufs=3) as wgu_pool,
            tc.tile_pool(name="wgp", bufs=1) as wgu_pre_pool,
            tc.tile_pool(name="ht", bufs=FT) as ht_pool,
            tc.tile_pool(name="wdr", bufs=1) as wd_pool,
            tc.tile_pool(name="act", bufs=2) as act_pool,
            tc.tile_pool(name="out", bufs=2) as out_pool,
            tc.tile_pool(name="scp", bufs=2) as sc_pool,
            tc.tile_pool(name="ps1", bufs=1, space="PSUM") as ps1_pool,
            tc.tile_pool(name="ps2", bufs=6, space="PSUM") as ps2_pool,
        ):
            # Warm-up: keep TensorE busy while the first tiles stream in, so
            # the HAM clock gate reaches full speed before real matmuls start.
            wsrc = warm_pool.tile([P, 256], bf16, name="wsrc")
            nc.vector.memset(wsrc[:], 0.0)
            wps = ps1_pool.tile([P, 512], fp32, name="psg")
            for wi in range(N_WARMUP):
                nc.tensor.matmul(
                    wps[:, :256],
                    wsrc[:, :P],
                    wsrc[:],
                    start=(wi == 0),
                    stop=(wi == N_WARMUP - 1),
                )

            # prefetch state: (wgt/wut tiles by ft, xg, sct) per group
            pre = {}

            def issue_group_loads(gi):
                g0, gc = groups[gi]
                gsub = gc // P
                npre = min(N_PRE_FT, FT)
                # critical-path order: first f-tile's weights (sync queue)
                # race the token slab (gpsimd queue); remaining prefetched
                # f-tiles follow as one block DMA
                block = wgu_pre_pool.tile([P, npre, DT, 2, P], bf16, name="wgup")
                nc.sync.dma_start(out=block[:, 0], in_=wgu_ap[0])
                xg = xg_pool.tile([P, DT, gc], bf16, name="xg")
                nc.gpsimd.dma_start(
                    out=xg[:],
                    in_=xT_ap[:, DT * g0 : DT * (g0 + gc)].rearrange(
                        "p (dt c) -> p dt c", c=gc
                    ),
                )
                sct = sc_pool.tile([P, GS], fp32, name="sct")
                nc.gpsimd.dma_start(
                    out=sct[:, :gsub], in_=sc_ap[:, g0 // P : g0 // P + gsub]
                )
                if npre > 1:
                    nc.sync.dma_start(
                        out=block[:, 1:npre],
                        in_=wgu_ap[1:npre].rearrange("f p d t j -> p f d t j"),
                    )
                wgu = {ft: block[:, ft] for ft in range(npre)}
                pre[gi] = (wgu, xg, sct)

            issue_group_loads(0)
            wd_res = wd_pool.tile([P, FT, D], bf16, name="wdr")

            for gi, (g0, gc) in enumerate(groups):
                gsub = gc // P
                batches = batch_plan(gsub, False)
                batches_last_d0 = batch_plan(gsub, gi == n_groups - 1)

                wgu_pre, xg, sct = pre.pop(gi)

                # equal-width chunks: keep every chunk >=320 so the fixed
                # per-matmul cost stays hidden
                n_ch = (gc + 511) // 512
                base_w, extra = divmod(gc, n_ch)
                chunks = []
                c0 = 0
                for ci in range(n_ch):
                    cw = base_w + (1 if ci < extra else 0)
                    chunks.append((c0, cw))
                    c0 += cw

                # ---- stage 1: hT[f, c] = silu(wg.T x) * (wu.T x) ----
                ht_tiles = []
                for ft in range(FT):
                    if ft in wgu_pre:
                        wgut = wgu_pre.pop(ft)
                    else:
                        wgut = wgu_pool.tile([P, DT, 2, P], bf16, name="wgut")
                        nc.sync.dma_start(out=wgut[:], in_=wgu_ap[ft])
                    if gi == 0 and 1 <= ft <= 4:
                        # wd is small in bf16 (8.4 MB): stream it once into
                        # SBUF on the otherwise-idle scalar queue (in
                        # quarters, so stage 2's first f-tiles don't wait on
                        # the whole transfer); never touches HBM again
                        q8 = FT // 4
                        f0 = (ft - 1) * q8
                        nc.scalar.dma_start(
                            out=wd_res[:, f0 : f0 + q8, :],
                            in_=wd_ap[:, f0 : f0 + q8, :],
                        )
                    if ft == (24 if gc >= 640 else 12) and gi + 1 < n_groups:
                        # queue the next group's token slab + first weights
                        # behind the remaining stage-1 loads: they land
                        # during this group's stage 2
                        issue_group_loads(gi + 1)
                    ht = ht_pool.tile([P, G_MAX], bf16, name="ht")
                    ht_tiles.append(ht)
                    for c0, cw in chunks:
                        psg = ps1_pool.tile([P, 512], fp32, name="psg")
                        for dt_i in range(DT):
                            nc.tensor.matmul(
                                psg[:, :cw],
                                wgut[:, dt_i, 0, :],
                                xg[:, dt_i, c0 : c0 + cw],
                                start=(dt_i == 0),
                                stop=(dt_i == DT - 1),
                            )
                        psu = ps1_pool.tile([P, 512], fp32, name="psu")
                        for dt_i in range(DT):
                            nc.tensor.matmul(
                                psu[:, :cw],
                                wgut[:, dt_i, 1, :],
                                xg[:, dt_i, c0 : c0 + cw],
                                start=(dt_i == 0),
                                stop=(dt_i == DT - 1),
                            )
                        sil = act_pool.tile([P, 512], fp32, name="sil")
                        nc.scalar.activation(sil[:, :cw], psg[:, :cw], silu_fn)
                        nc.vector.tensor_tensor(
                            out=ht[:, c0 : c0 + cw],
                            in0=sil[:, :cw],
                            in1=psu[:, :cw],
                            op=mult_op,
                        )

                # ---- stage 2: y[c, d] = sum_f hT[f, c] * wd[f, d], scaled ----
                for d0 in range(0, D_MODEL, 512):
                    cur_batches = batches_last_d0 if d0 == 512 else batches
                    for bi, cs_list in enumerate(cur_batches):
                        ps_out = [
                            ps2_pool.tile([P, 512], fp32, name="pso") for _ in cs_list
                        ]
                        for ft in range(FT):
                            for i, cs in enumerate(cs_list):
                                nc.tensor.matmul(
                                    ps_out[i][:],
                                    ht_tiles[ft][:, cs * P : (cs + 1) * P],
                                    wd_res[:, ft, d0 : d0 + 512],
                                    start=(ft == 0),
                                    stop=(ft == FT - 1),
                                )
                        for i, cs in enumerate(cs_list):
                            ot = out_pool.tile([P, 512], fp32, name="ot")
                            if i % 2 == 0:
                                nc.vector.tensor_scalar_mul(
                                    ot[:], ps_out[i][:], sct[:, cs : cs + 1]
                                )
                            else:
                                # spread evictions across engines so the bank
                                # ring frees faster at d0 boundaries
                                nc.scalar.activation(
                                    ot[:],
                                    ps_out[i][:],
                                    mybir.ActivationFunctionType.Copy,
                                    scale=sct[:, cs : cs + 1],
                                )
                            r0 = g0 + cs * P
                            nc.gpsimd.dma_start(
                                out=y_ap[r0 : r0 + P, d0 : d0 + 512], in_=ot[:]
                            )
    nc.compile()
    return nc


def _get_program(C):
    if C not in _programs:
        _programs[C] = _build_program(C)
    return _programs[C]


def _route(xf, router_w):
    """Host router, float64 (all f32 evaluation orders agree on this input's
    top-2 sets; f64 is the stable reference ranking). Mirrors
    softmax -> top_k(2) -> renormalize from the reference."""
    logits = xf.astype(np.float64) @ router_w.astype(np.float64).T
    logits -= logits.max(axis=-1, keepdims=True)
    sm = np.exp(logits)
    sm /= sm.sum(axis=-1, keepdims=True)
    top = np.argsort(-sm, axis=-1, kind="stable")[:, :TOP_K]
    tsc = np.take_along_axis(sm, top, axis=1)
    tsc = tsc / tsc.sum(axis=-1, keepdims=True)
    return top, tsc


def kernel(x, router_w, w_gate, w_up, w_down):
    global LAST_EXEC_NS
    from concourse.bass_utils import run_bass_kernel_spmd

    trace = os.environ.get("MOE_TRACE", "0") == "1"
    if trace:
        _ensure_axon_hooks()

    x = np.asarray(x, dtype=np.float32)
    router_w = np.asarray(router_w, dtype=np.float32)
    w_gate = np.asarray(w_gate, dtype=np.float32)
    w_up = np.asarray(w_up, dtype=np.float32)
    w_down = np.asarray(w_down, dtype=np.float32)

    B, T, D = x.shape
    N = B * T
    F = D_FF
    FT, DT = F // P, D // P
    xf = np.ascontiguousarray(x.reshape(N, D))

    top, tsc = _route(xf, router_w)

    tok_rows = []
    tok_wts = []
    for e in range(N_EXPERTS):
        mask = top == e
        rows = np.nonzero(mask.any(axis=1))[0]
        wts = tsc[mask].astype(np.float32)
        tok_rows.append(rows)
        tok_wts.append(wts)

    cmax = max(max(len(r) for r in tok_rows), 1)
    C = max(((cmax + P - 1) // P) * P, 256)

    nc = _get_program(C)

    # pre-tile weights host-side (bf16) so device DMAs are contiguous and
    # gate+up arrive in one DMA per f-tile:
    #   wguT[e][ft, p, dt, 0/1, j] = w_gate/w_up[e, dt*128+p, ft*128+j]
    #   wdT[e][p, ft, d]           = w_down[e, ft*128+p, d]
    wg16 = w_gate.astype(BF16).reshape(N_EXPERTS, DT, P, FT, P).transpose(0, 3, 2, 1, 4)
    wu16 = w_up.astype(BF16).reshape(N_EXPERTS, DT, P, FT, P).transpose(0, 3, 2, 1, 4)
    wgu16 = np.ascontiguousarray(np.stack([wg16, wu16], axis=4))
    wd16 = np.ascontiguousarray(
        w_down.astype(BF16).reshape(N_EXPERTS, FT, P, D).transpose(0, 2, 1, 3)
    )

    sizes = _group_sizes(C)
    in_maps = []
    for e in range(N_EXPERTS):
        rows = tok_rows[e]
        xg = np.zeros((C, D), np.float32)
        xg[: len(rows)] = xf[rows]
        # group-major flat layout: per group a contiguous [dt, c] slab per
        # partition row
        x16 = xg.astype(BF16).reshape(C, DT, P)
        parts = []
        g0 = 0
        for gc in sizes:
            parts.append(x16[g0 : g0 + gc].transpose(2, 1, 0).reshape(P, DT * gc))
            g0 += gc
        xflat = np.ascontiguousarray(np.concatenate(parts, axis=1))
        scv = np.zeros((C,), np.float32)
        scv[: len(rows)] = tok_wts[e]
        in_maps.append(
            {
                "xT": xflat,
                "wgu": wgu16[e],
                "wd": wd16[e],
                "sc": np.ascontiguousarray(scv.reshape(C // P, P).T),
            }
        )

    res = run_bass_kernel_spmd(nc, in_maps, list(range(N_EXPERTS)), trace=trace)
    if trace:
        LAST_EXEC_NS = res.exec_time_ns

    out = np.zeros((N, D), np.float32)
    for e in range(N_EXPERTS):
        rows = tok_rows[e]
        out[rows] += res.results[e]["y"][: len(rows)]
    return out.reshape(B, T, D)
